# revision 10
# baseline (speedup 1.0000x reference)
"""Trainium2 Bass kernel for nn_Decoder_74938589381001.

Strategy (8 NeuronCores, SPMD single program):
  - Branch-parallel data split: cores 0-3 run the u-branch on 8 samples each,
    cores 4-7 run the l-branch on 8 samples each (branch weights are input
    data, so the program is identical across cores).
  - All convs are lowered to fp16 matmuls (fp32 PSUM accumulation, fp32
    residual stream in SBUF). Upsample+conv fuses to a pair of stride-2 k2
    convs. The degenerate batch-0 GCN mix is resolved on the host by
    linearity: every core computes BOTH final convs, and the host averages
    the sample-0 outputs of core 0 and core 4.
"""

import os
import numpy as np

# The bass kernel executes through the axon PJRT proxy; a cpu-pinned JAX env
# (sometimes set for reference runs) would hide the neuron devices.
if os.environ.get("JAX_PLATFORMS", "").strip().lower() == "cpu":
    os.environ.pop("JAX_PLATFORMS")

W = 512
B = 8            # samples per core
T0 = 64
DILS = (9, 3, 1)
NCORES = 8
TC = 64          # time-columns per matmul group (B*TC = 512 free dim)
U_FEATS = 156
L_FEATS = 107
GCN_OUT = 156
GCN_PAD = 256    # gcn rows padded so final convs contract 2 full chunks


def _layer_list():
    L = [("in", dict(n_ic=4, ocw=(128,) * 4, taps=(-1, 0, 1), dil=1))]
    for b in range(3):
        for ri, d in enumerate(DILS):
            L.append((f"b{b}r{ri}c1", dict(n_ic=4, ocw=(128,) * 4, taps=(-1, 0, 1), dil=d)))
            L.append((f"b{b}r{ri}c2", dict(n_ic=4, ocw=(128,) * 4, taps=(0,), dil=1)))
        L.append((f"up{b}e", dict(n_ic=4, ocw=(128,) * 4, taps=(-1, 0), dil=1)))
        L.append((f"up{b}o", dict(n_ic=4, ocw=(128,) * 4, taps=(0, 1), dil=1)))
    L += [
        ("out1", dict(n_ic=4, ocw=(128,) * 4, taps=(-1, 0, 1), dil=1)),
        ("out2", dict(n_ic=4, ocw=(128,) * 4, taps=(-1, 0, 1), dil=1)),
        ("gcn", dict(n_ic=4, ocw=(128, 128), taps=(0,), dil=1)),
        ("uconv", dict(n_ic=2, ocw=(128, 28), taps=(-1, 0, 1), dil=1)),
        ("lconv", dict(n_ic=2, ocw=(128, 28), taps=(-1, 0, 1), dil=1)),
    ]
    return L


LAYERS = _layer_list()
LSPEC = dict(LAYERS)


def _offsets():
    woff, boff = {}, {}
    wo = bo = 0
    for name, s in LAYERS:
        woff[name] = wo
        boff[name] = bo
        wo += len(s["ocw"]) * s["n_ic"] * len(s["taps"]) * 128
        bo += len(s["ocw"])
    return woff, boff, wo, bo


WOFF, BOFF, NWCOLS, NBCOLS = _offsets()


# ----------------------------------------------------------------------------
# Host-side packing
# ----------------------------------------------------------------------------

def _blocks(Wm, ocw_list, n_ic):
    """Wm: [Cout, Cin<=n_ic*128, K] fp32 -> lhsT blocks [slots, 128, 128] f16.

    slot index = (oc*n_ic + ic)*K + k; block[r, c] = Wm[oc0+c, ic0+r, k].
    """
    Cout, Cin, K = Wm.shape
    out = np.zeros((len(ocw_list) * n_ic * K, 128, 128), np.float16)
    s = 0
    for oi, ocw in enumerate(ocw_list):
        for ic in range(n_ic):
            icw = min(128, Cin - ic * 128)
            for k in range(K):
                blk = Wm[oi * 128:oi * 128 + ocw, ic * 128:ic * 128 + icw, k]
                out[s, :icw, :ocw] = blk.T.astype(np.float16)
                s += 1
    return out


def _bias_cols(bv, ocw_list):
    cols = np.zeros((128, len(ocw_list)), np.float32)
    for oi, ocw in enumerate(ocw_list):
        seg = bv[oi * 128:oi * 128 + ocw]
        cols[:len(seg), oi] = seg
    return cols


def _pack_core_weights(bp, gcn_w, gcn_b, uconv_w, uconv_b, lconv_w, lconv_b):
    """bp: branch params dict. Returns (wb [128, NWCOLS] f16, bb [128, NBCOLS] f32)."""
    wblocks, bcols = [], []

    def add(name, Wm, bv):
        s = LSPEC[name]
        assert Wm.shape[2] == len(s["taps"]), name
        wblocks.append(_blocks(Wm, s["ocw"], s["n_ic"]))
        bcols.append(_bias_cols(bv, s["ocw"]))

    add("in", bp["in_w"], bp["in_b"])
    for b in range(3):
        blk = bp["blocks"][b]
        for ri in range(3):
            rb = blk["res"][ri]
            add(f"b{b}r{ri}c1", rb["w1"], rb["b1"])
            add(f"b{b}r{ri}c2", rb["w2"], rb["b2"])
        w = blk["up_w"]
        We = np.stack([w[:, :, 0], w[:, :, 1] + w[:, :, 2]], axis=2)  # taps -1, 0
        Wo = np.stack([w[:, :, 0] + w[:, :, 1], w[:, :, 2]], axis=2)  # taps 0, +1
        add(f"up{b}e", We, blk["up_b"])
        add(f"up{b}o", Wo, blk["up_b"])
    add("out1", bp["out1_w"], bp["out1_b"])
    add("out2", bp["out2_w"], bp["out2_b"])

    gw = np.zeros((GCN_PAD, W, 1), np.float32)
    gw[:GCN_OUT, :, 0] = gcn_w
    gb = np.zeros(GCN_PAD, np.float32)
    gb[:GCN_OUT] = gcn_b
    add("gcn", gw, gb)

    uw = np.zeros((U_FEATS, GCN_PAD, 3), np.float32)
    uw[:, :GCN_OUT, :] = uconv_w
    add("uconv", uw, uconv_b)
    lw = np.zeros((U_FEATS, GCN_PAD, 3), np.float32)
    lw[:L_FEATS, :GCN_OUT, :] = lconv_w
    lb = np.zeros(U_FEATS, np.float32)
    lb[:L_FEATS] = lconv_b
    add("lconv", lw, lb)

    blocks = np.concatenate(wblocks, axis=0)          # [slots, 128, 128]
    assert blocks.shape[0] * 128 == NWCOLS
    wb = np.ascontiguousarray(blocks.transpose(1, 0, 2).reshape(128, NWCOLS))
    bb = np.ascontiguousarray(np.concatenate(bcols, axis=1))
    assert bb.shape == (128, NBCOLS)
    return wb, bb


def _pack_x(xs):
    """xs: [B, 512, 64] fp32 -> [128, 4, B, 66] f16 with zeroed t-halo."""
    a = np.zeros((128, 4, B, T0 + 2), np.float16)
    xr = np.asarray(xs, np.float32).reshape(B, 4, 128, T0).astype(np.float16)
    a[:, :, :, 1:T0 + 1] = xr.transpose(2, 1, 0, 3)
    return a


# ----------------------------------------------------------------------------
# BIR post-pass: walrus rejects instructions with more than a couple of sync
# waits; split excess waits onto same-engine NOPs inserted just before.
# ----------------------------------------------------------------------------

def _split_excess_waits(nc, mybir, max_waits=1):
    ctr = 0
    for f in nc.m.functions:
        for blk in f.blocks:
            insts = blk.instructions
            i = 0
            while i < len(insts):
                inst = insts[i]
                si = inst.sync_info
                waits = list(si.on_wait) if si and si.on_wait else []
                if len(waits) > max_waits:
                    keep = waits[-max_waits:]
                    extra = waits[:-max_waits]
                    pos = i
                    for j in range(0, len(extra), max_waits):
                        chunk = extra[j:j + max_waits]
                        nop = mybir.InstNoOp(name=f"waitnop_{ctr}", ins=[], outs=[])
                        ctr += 1
                        nop.engine = inst.engine
                        nop.sync_info = mybir.SyncInfo(on_wait=list(chunk), on_update=[])
                        insts.insert(pos, nop)
                        pos += 1
                        i += 1
                    si.on_wait = list(keep)
                i += 1
    return ctr


# ----------------------------------------------------------------------------
# Device program
# ----------------------------------------------------------------------------

_NC_CACHE = []


def _build_nc():
    import concourse.bass as bass
    import concourse.tile as tile
    from concourse import mybir

    f16 = mybir.dt.float16
    f32 = mybir.dt.float32
    AF = mybir.ActivationFunctionType
    ADD = mybir.AluOpType.add

    nc = bass.Bass()
    xin = nc.declare_dram_parameter("xin", [128, 4, B, T0 + 2], f16, isOutput=False)
    wb = nc.declare_dram_parameter("wb", [128, NWCOLS], f16, isOutput=False)
    bbp = nc.declare_dram_parameter("bb", [128, NBCOLS], f32, isOutput=False)
    out = nc.declare_dram_parameter("out", [2, U_FEATS, B, W], f32, isOutput=True)

    with tile.TileContext(nc) as tc:
        from contextlib import ExitStack
        with ExitStack() as ctx:
            wpool = ctx.enter_context(tc.tile_pool(name="wpool", bufs=3))
            xpool = ctx.enter_context(tc.tile_pool(name="xpool", bufs=5))
            rpool = ctx.enter_context(tc.tile_pool(name="rpool", bufs=5))
            rrpool = ctx.enter_context(tc.tile_pool(name="rrpool", bufs=4))
            opool = ctx.enter_context(tc.tile_pool(name="opool", bufs=2))
            spool = ctx.enter_context(tc.tile_pool(name="spool", bufs=1))
            ppool = ctx.enter_context(tc.tile_pool(name="ppool", bufs=6, space="PSUM"))

            bias_sb = spool.tile([128, NBCOLS], f32, name="bias_sb", tag="bias")
            nc.sync.dma_start(out=bias_sb[:, :], in_=bbp[:, :])

            def load_w(name):
                s = LSPEC[name]
                cols = len(s["ocw"]) * s["n_ic"] * len(s["taps"]) * 128
                wsb = wpool.tile([128, cols], f16, name=f"w_{name}", tag="w")
                nc.sync.dma_start(out=wsb[:, :], in_=wb[:, WOFF[name]:WOFF[name] + cols])
                return wsb

            def conv(name, src, src_halo, T_in, *, dst=None, dst_halo=0,
                     act=None, stt_into=None, stride=1, suboff=0, groups=None,
                     dma_to=None):
                """One conv layer. src/dst: lists of [128, B, Tp] tiles.

                groups = number of TC-wide input column groups (T_in // TC).
                stride=2 writes outputs at dst positions dst_halo+suboff+2*t.
                """
                s = LSPEC[name]
                wsb = load_w(name)
                ng = groups if groups is not None else T_in // TC
                ntap = len(s["taps"])
                nmm = s["n_ic"] * ntap
                for g in range(ng):
                    for oi, ocw in enumerate(s["ocw"]):
                        ps = ppool.tile([128, B, TC], f32, name=f"ps_{name}_{g}_{oi}", tag="ps")
                        i = 0
                        for ic in range(s["n_ic"]):
                            for ki, tap in enumerate(s["taps"]):
                                slot = (oi * s["n_ic"] + ic) * ntap + ki
                                p0 = src_halo + g * TC + tap * s["dil"]
                                rhs = src[ic][:, :, p0:p0 + TC]
                                nc.tensor.matmul(
                                    ps[0:ocw, :, :],
                                    wsb[:, slot * 128: slot * 128 + ocw],
                                    rhs,
                                    start=(i == 0), stop=(i == nmm - 1),
                                )
                                i += 1
                        b_ap = bias_sb[0:ocw, BOFF[name] + oi: BOFF[name] + oi + 1]
                        if stt_into is not None:
                            xs = stt_into[oi][:, :, g * TC:(g + 1) * TC]
                            nc.vector.scalar_tensor_tensor(
                                out=xs, in0=ps[0:ocw, :, :], scalar=b_ap, in1=xs,
                                op0=ADD, op1=ADD)
                        else:
                            if stride == 1:
                                oap = dst[oi][0:ocw, :, dst_halo + g * TC: dst_halo + (g + 1) * TC]
                            else:
                                a = dst_halo + suboff + 2 * g * TC
                                oap = dst[oi][0:ocw, :, a:a + 2 * TC - 1:2]
                            nc.scalar.activation(oap, ps[0:ocw, :, :], act, bias=b_ap)
                            if dma_to is not None and g % 2 == 1:
                                c0 = (g - 1) * TC
                                nc.sync.dma_start(
                                    out=dma_to[oi][:, :, c0:c0 + 2 * TC],
                                    in_=dst[oi][0:ocw, :, c0:c0 + 2 * TC])

            def alloc4(pool, T, halo, dt, nm, tag, zero_halo=True, n=4):
                ts = []
                for c in range(n):
                    t = pool.tile([128, B, T + 2 * halo], dt, name=f"{nm}{c}", tag=tag)
                    if halo and zero_halo:
                        nc.vector.memset(t[:, :, 0:halo], 0.0)
                        nc.vector.memset(t[:, :, halo + T:T + 2 * halo], 0.0)
                    ts.append(t)
                return ts

            # ---- input ----
            x0 = []
            for c in range(4):
                t = rpool.tile([128, B, T0 + 2], f16, name=f"x0_{c}", tag="r")
                nc.sync.dma_start(out=t[:, :, :], in_=xin[:, c, :, :])
                x0.append(t)

            x = alloc4(xpool, T0, 0, f32, "xin_conv", "x")
            conv("in", x0, 1, T0, dst=x, act=AF.Relu)

            # ---- res blocks + upsample ----
            T = T0
            for b in range(3):
                for ri, d in enumerate(DILS):
                    r = alloc4(rpool, T, d, f16, f"r{b}{ri}", "r")
                    for c in range(4):
                        nc.scalar.activation(r[c][:, :, d:d + T], x[c][:, :, :], AF.Relu)
                    rr = alloc4(rrpool, T, 0, f16, f"rr{b}{ri}", "rr")
                    conv(f"b{b}r{ri}c1", r, d, T, dst=rr, act=AF.Relu)
                    conv(f"b{b}r{ri}c2", rr, 0, T, stt_into=x)
                # upsample fused into even/odd stride-2 convs
                rup = alloc4(rpool, T, 1, f16, f"rup{b}", "r")
                for c in range(4):
                    nc.vector.tensor_copy(rup[c][:, :, 1:1 + T], x[c][:, :, :])
                last = (b == 2)
                T2 = 2 * T
                if last:
                    xn = alloc4(xpool, T2, 1, f16, f"xup{b}", "x")
                else:
                    xn = alloc4(xpool, T2, 0, f32, f"xup{b}", "x")
                conv(f"up{b}e", rup, 1, T, dst=xn, dst_halo=1 if last else 0,
                     act=AF.Identity, stride=2, suboff=0)
                conv(f"up{b}o", rup, 1, T, dst=xn, dst_halo=1 if last else 0,
                     act=AF.Identity, stride=2, suboff=1)
                x = xn
                T = T2

            # ---- out path (T=512), x is f16 with halo 1 ----
            r4 = alloc4(rpool, T, 1, f16, "r4", "r")
            conv("out1", x, 1, T, dst=r4, dst_halo=1, act=AF.Relu)
            y2 = alloc4(rrpool, T, 0, f16, "y2", "rr")
            conv("out2", r4, 1, T, dst=y2, act=AF.Identity)

            h = alloc4(rpool, T, 1, f16, "h", "r", n=2)
            conv("gcn", y2, 0, T, dst=h, dst_halo=1, act=AF.Identity)

            rows = (slice(0, 128), slice(128, U_FEATS))
            for cv, name in enumerate(("uconv", "lconv")):
                ots = [opool.tile([128, B, W], f32, name=f"o_{name}{oi}", tag="out")
                       for oi in range(2)]
                conv(name, h, 1, T, dst=ots, act=AF.Identity,
                     dma_to=[out[cv, rows[0], :, :], out[cv, rows[1], :, :]])

    _split_excess_waits(nc, mybir)
    return nc


def _get_nc():
    if not _NC_CACHE:
        _NC_CACHE.append(_build_nc())
    return _NC_CACHE[0]


# ----------------------------------------------------------------------------
# Entry point
# ----------------------------------------------------------------------------

def kernel(ux, lx, ubody_params, lbody_params, gcn_w, gcn_b,
           uconv_w, uconv_b, lconv_w, lconv_b):
    from concourse.bass_utils import run_bass_kernel_spmd

    def tonp(t):
        return np.asarray(t, np.float32)

    def tree(p):
        if isinstance(p, dict):
            return {k: tree(v) for k, v in p.items()}
        if isinstance(p, list):
            return [tree(v) for v in p]
        return tonp(p)

    ux = tonp(ux)
    lx = tonp(lx)
    ub, lb = tree(ubody_params), tree(lbody_params)
    gcn_w, gcn_b = tonp(gcn_w), tonp(gcn_b)
    uconv_w, uconv_b = tonp(uconv_w), tonp(uconv_b)
    lconv_w, lconv_b = tonp(lconv_w), tonp(lconv_b)

    nbatch = ux.shape[0]
    per = nbatch // 4  # samples per core (u on cores 0-3, l on 4-7)
    assert per == B

    wb_u, bb_u = _pack_core_weights(ub, gcn_w, gcn_b, uconv_w, uconv_b, lconv_w, lconv_b)
    wb_l, bb_l = _pack_core_weights(lb, gcn_w, gcn_b, uconv_w, uconv_b, lconv_w, lconv_b)

    in_maps = []
    for core in range(NCORES):
        branch_u = core < 4
        i0 = (core % 4) * B
        xs = ux[i0:i0 + B] if branch_u else lx[i0:i0 + B]
        in_maps.append({
            "xin": _pack_x(xs),
            "wb": wb_u if branch_u else wb_l,
            "bb": bb_u if branch_u else bb_l,
        })

    nc = _get_nc()
    res = run_bass_kernel_spmd(nc, in_maps, list(range(NCORES)))
    outs = [res.results[c]["out"] for c in range(NCORES)]

    ux_out = np.empty((nbatch, U_FEATS, W), np.float32)
    lx_out = np.empty((nbatch, L_FEATS, W), np.float32)
    for i in range(4):
        for bidx in range(B):
            s = i * B + bidx
            ux_out[s] = outs[i][0, :, bidx, :]
            lx_out[s] = outs[i + 4][1, :L_FEATS, bidx, :]
    # batch-0 GCN mix by linearity: uconv(0.5*(h_u0+h_l0)) = 0.5*(uconv(h_u0)+uconv(h_l0))
    ux_out[0] = 0.5 * (outs[0][0, :, 0, :] + outs[4][0, :, 0, :])
    lx_out[0] = 0.5 * (outs[0][1, :L_FEATS, 0, :] + outs[4][1, :L_FEATS, 0, :])
    return ux_out, lx_out


# revision 11
# speedup vs baseline: 1.0023x; 1.0023x over previous
"""Trainium2 Bass kernel for nn_Decoder_74938589381001.

Strategy (8 NeuronCores, SPMD single program):
  - Branch-parallel data split: cores 0-3 run the u-branch on 8 samples each,
    cores 4-7 run the l-branch on 8 samples each (branch weights are input
    data, so the program is identical across cores).
  - All convs are lowered to fp16 matmuls (fp32 PSUM accumulation, fp32
    residual stream in SBUF). Upsample+conv fuses to a pair of stride-2 k2
    convs. The degenerate batch-0 GCN mix is resolved on the host by
    linearity: every core computes BOTH final convs, and the host averages
    the sample-0 outputs of core 0 and core 4.
"""

import os
import numpy as np

# The bass kernel executes through the axon PJRT proxy; a cpu-pinned JAX env
# (sometimes set for reference runs) would hide the neuron devices.
if os.environ.get("JAX_PLATFORMS", "").strip().lower() == "cpu":
    os.environ.pop("JAX_PLATFORMS")

W = 512
B = 8            # samples per core
T0 = 64
DILS = (9, 3, 1)
NCORES = 8
TC = 64          # time-columns per matmul group (B*TC = 512 free dim)
U_FEATS = 156
L_FEATS = 107
GCN_OUT = 156
GCN_PAD = 256    # gcn rows padded so final convs contract 2 full chunks


def _layer_list():
    L = [("in", dict(n_ic=4, ocw=(128,) * 4, taps=(-1, 0, 1), dil=1))]
    for b in range(3):
        for ri, d in enumerate(DILS):
            L.append((f"b{b}r{ri}c1", dict(n_ic=4, ocw=(128,) * 4, taps=(-1, 0, 1), dil=d)))
            L.append((f"b{b}r{ri}c2", dict(n_ic=4, ocw=(128,) * 4, taps=(0,), dil=1)))
        L.append((f"up{b}e", dict(n_ic=4, ocw=(128,) * 4, taps=(-1, 0), dil=1)))
        L.append((f"up{b}o", dict(n_ic=4, ocw=(128,) * 4, taps=(0, 1), dil=1)))
    L += [
        ("out1", dict(n_ic=4, ocw=(128,) * 4, taps=(-1, 0, 1), dil=1)),
        ("out2", dict(n_ic=4, ocw=(128,) * 4, taps=(-1, 0, 1), dil=1)),
        ("gcn", dict(n_ic=4, ocw=(128, 128), taps=(0,), dil=1)),
        ("uconv", dict(n_ic=2, ocw=(128, 28), taps=(-1, 0, 1), dil=1)),
        ("lconv", dict(n_ic=2, ocw=(128, 28), taps=(-1, 0, 1), dil=1)),
    ]
    return L


LAYERS = _layer_list()
LSPEC = dict(LAYERS)


def _offsets():
    woff, boff = {}, {}
    wo = bo = 0
    for name, s in LAYERS:
        woff[name] = wo
        boff[name] = bo
        wo += len(s["ocw"]) * s["n_ic"] * len(s["taps"]) * 128
        bo += len(s["ocw"])
    return woff, boff, wo, bo


WOFF, BOFF, NWCOLS, NBCOLS = _offsets()


# ----------------------------------------------------------------------------
# Host-side packing
# ----------------------------------------------------------------------------

def _blocks(Wm, ocw_list, n_ic):
    """Wm: [Cout, Cin<=n_ic*128, K] fp32 -> lhsT blocks [slots, 128, 128] f16.

    slot index = (oc*n_ic + ic)*K + k; block[r, c] = Wm[oc0+c, ic0+r, k].
    """
    Cout, Cin, K = Wm.shape
    out = np.zeros((len(ocw_list) * n_ic * K, 128, 128), np.float16)
    s = 0
    for oi, ocw in enumerate(ocw_list):
        for ic in range(n_ic):
            icw = min(128, Cin - ic * 128)
            for k in range(K):
                blk = Wm[oi * 128:oi * 128 + ocw, ic * 128:ic * 128 + icw, k]
                out[s, :icw, :ocw] = blk.T.astype(np.float16)
                s += 1
    return out


def _bias_cols(bv, ocw_list):
    cols = np.zeros((128, len(ocw_list)), np.float32)
    for oi, ocw in enumerate(ocw_list):
        seg = bv[oi * 128:oi * 128 + ocw]
        cols[:len(seg), oi] = seg
    return cols


def _pack_core_weights(bp, gcn_w, gcn_b, uconv_w, uconv_b, lconv_w, lconv_b):
    """bp: branch params dict. Returns (wb [128, NWCOLS] f16, bb [128, NBCOLS] f32)."""
    wblocks, bcols = [], []

    def add(name, Wm, bv):
        s = LSPEC[name]
        assert Wm.shape[2] == len(s["taps"]), name
        wblocks.append(_blocks(Wm, s["ocw"], s["n_ic"]))
        bcols.append(_bias_cols(bv, s["ocw"]))

    add("in", bp["in_w"], bp["in_b"])
    for b in range(3):
        blk = bp["blocks"][b]
        for ri in range(3):
            rb = blk["res"][ri]
            add(f"b{b}r{ri}c1", rb["w1"], rb["b1"])
            add(f"b{b}r{ri}c2", rb["w2"], rb["b2"])
        w = blk["up_w"]
        We = np.stack([w[:, :, 0], w[:, :, 1] + w[:, :, 2]], axis=2)  # taps -1, 0
        Wo = np.stack([w[:, :, 0] + w[:, :, 1], w[:, :, 2]], axis=2)  # taps 0, +1
        add(f"up{b}e", We, blk["up_b"])
        add(f"up{b}o", Wo, blk["up_b"])
    add("out1", bp["out1_w"], bp["out1_b"])
    add("out2", bp["out2_w"], bp["out2_b"])

    gw = np.zeros((GCN_PAD, W, 1), np.float32)
    gw[:GCN_OUT, :, 0] = gcn_w
    gb = np.zeros(GCN_PAD, np.float32)
    gb[:GCN_OUT] = gcn_b
    add("gcn", gw, gb)

    uw = np.zeros((U_FEATS, GCN_PAD, 3), np.float32)
    uw[:, :GCN_OUT, :] = uconv_w
    add("uconv", uw, uconv_b)
    lw = np.zeros((U_FEATS, GCN_PAD, 3), np.float32)
    lw[:L_FEATS, :GCN_OUT, :] = lconv_w
    lb = np.zeros(U_FEATS, np.float32)
    lb[:L_FEATS] = lconv_b
    add("lconv", lw, lb)

    blocks = np.concatenate(wblocks, axis=0)          # [slots, 128, 128]
    assert blocks.shape[0] * 128 == NWCOLS
    wb = np.ascontiguousarray(blocks.transpose(1, 0, 2).reshape(128, NWCOLS))
    bb = np.ascontiguousarray(np.concatenate(bcols, axis=1))
    assert bb.shape == (128, NBCOLS)
    return wb, bb


def _pack_x(xs):
    """xs: [B, 512, 64] fp32 -> [128, 4, B, 66] f16 with zeroed t-halo."""
    a = np.zeros((128, 4, B, T0 + 2), np.float16)
    xr = np.asarray(xs, np.float32).reshape(B, 4, 128, T0).astype(np.float16)
    a[:, :, :, 1:T0 + 1] = xr.transpose(2, 1, 0, 3)
    return a


# ----------------------------------------------------------------------------
# BIR post-pass: walrus rejects instructions with more than a couple of sync
# waits; split excess waits onto same-engine NOPs inserted just before.
# ----------------------------------------------------------------------------

def _split_excess_waits(nc, mybir, max_waits=1):
    ctr = 0
    for f in nc.m.functions:
        for blk in f.blocks:
            insts = blk.instructions
            i = 0
            while i < len(insts):
                inst = insts[i]
                si = inst.sync_info
                waits = list(si.on_wait) if si and si.on_wait else []
                if len(waits) > max_waits:
                    keep = waits[-max_waits:]
                    extra = waits[:-max_waits]
                    pos = i
                    for j in range(0, len(extra), max_waits):
                        chunk = extra[j:j + max_waits]
                        nop = mybir.InstNoOp(name=f"waitnop_{ctr}", ins=[], outs=[])
                        ctr += 1
                        nop.engine = inst.engine
                        nop.sync_info = mybir.SyncInfo(on_wait=list(chunk), on_update=[])
                        insts.insert(pos, nop)
                        pos += 1
                        i += 1
                    si.on_wait = list(keep)
                i += 1
    return ctr


# ----------------------------------------------------------------------------
# Device program
# ----------------------------------------------------------------------------

_NC_CACHE = []


def _build_nc():
    import concourse.bass as bass
    import concourse.tile as tile
    from concourse import mybir

    f16 = mybir.dt.float16
    f32 = mybir.dt.float32
    AF = mybir.ActivationFunctionType
    ADD = mybir.AluOpType.add

    nc = bass.Bass()
    xin = nc.declare_dram_parameter("xin", [128, 4, B, T0 + 2], f16, isOutput=False)
    wb = nc.declare_dram_parameter("wb", [128, NWCOLS], f16, isOutput=False)
    bbp = nc.declare_dram_parameter("bb", [128, NBCOLS], f32, isOutput=False)
    out = nc.declare_dram_parameter("out", [2, U_FEATS, B, W], f32, isOutput=True)

    with tile.TileContext(nc) as tc:
        from contextlib import ExitStack
        with ExitStack() as ctx:
            wpool = ctx.enter_context(tc.tile_pool(name="wpool", bufs=3))
            xpool = ctx.enter_context(tc.tile_pool(name="xpool", bufs=5))
            rpool = ctx.enter_context(tc.tile_pool(name="rpool", bufs=5))
            rrpool = ctx.enter_context(tc.tile_pool(name="rrpool", bufs=4))
            opool = ctx.enter_context(tc.tile_pool(name="opool", bufs=2))
            spool = ctx.enter_context(tc.tile_pool(name="spool", bufs=1))
            ppool = ctx.enter_context(tc.tile_pool(name="ppool", bufs=6, space="PSUM"))

            bias_sb = spool.tile([128, NBCOLS], f32, name="bias_sb", tag="bias")
            nc.sync.dma_start(out=bias_sb[:, :], in_=bbp[:, :])

            def load_w(name):
                s = LSPEC[name]
                cols = len(s["ocw"]) * s["n_ic"] * len(s["taps"]) * 128
                wsb = wpool.tile([128, cols], f16, name=f"w_{name}", tag="w")
                if name == "in":
                    # split the first conv's weight load so the PE can start on
                    # oc-chunk 0 before the whole layer's weights have landed
                    q = cols // len(s["ocw"])
                    for i in range(len(s["ocw"])):
                        nc.sync.dma_start(
                            out=wsb[:, i * q:(i + 1) * q],
                            in_=wb[:, WOFF[name] + i * q:WOFF[name] + (i + 1) * q])
                else:
                    nc.sync.dma_start(out=wsb[:, :], in_=wb[:, WOFF[name]:WOFF[name] + cols])
                return wsb

            def conv(name, src, src_halo, T_in, *, dst=None, dst_halo=0,
                     act=None, stt_into=None, stride=1, suboff=0, groups=None,
                     dma_to=None):
                """One conv layer. src/dst: lists of [128, B, Tp] tiles.

                groups = number of TC-wide input column groups (T_in // TC).
                stride=2 writes outputs at dst positions dst_halo+suboff+2*t.
                """
                s = LSPEC[name]
                wsb = load_w(name)
                ng = groups if groups is not None else T_in // TC
                ntap = len(s["taps"])
                nmm = s["n_ic"] * ntap
                for g in range(ng):
                    for oi, ocw in enumerate(s["ocw"]):
                        ps = ppool.tile([128, B, TC], f32, name=f"ps_{name}_{g}_{oi}", tag="ps")
                        i = 0
                        for ic in range(s["n_ic"]):
                            for ki, tap in enumerate(s["taps"]):
                                slot = (oi * s["n_ic"] + ic) * ntap + ki
                                p0 = src_halo + g * TC + tap * s["dil"]
                                rhs = src[ic][:, :, p0:p0 + TC]
                                nc.tensor.matmul(
                                    ps[0:ocw, :, :],
                                    wsb[:, slot * 128: slot * 128 + ocw],
                                    rhs,
                                    start=(i == 0), stop=(i == nmm - 1),
                                )
                                i += 1
                        b_ap = bias_sb[0:ocw, BOFF[name] + oi: BOFF[name] + oi + 1]
                        if stt_into is not None:
                            xs = stt_into[oi][:, :, g * TC:(g + 1) * TC]
                            nc.vector.scalar_tensor_tensor(
                                out=xs, in0=ps[0:ocw, :, :], scalar=b_ap, in1=xs,
                                op0=ADD, op1=ADD)
                        else:
                            if stride == 1:
                                oap = dst[oi][0:ocw, :, dst_halo + g * TC: dst_halo + (g + 1) * TC]
                            else:
                                a = dst_halo + suboff + 2 * g * TC
                                oap = dst[oi][0:ocw, :, a:a + 2 * TC - 1:2]
                            nc.scalar.activation(oap, ps[0:ocw, :, :], act, bias=b_ap)
                            if dma_to is not None and g % 2 == 1:
                                c0 = (g - 1) * TC
                                nc.sync.dma_start(
                                    out=dma_to[oi][:, :, c0:c0 + 2 * TC],
                                    in_=dst[oi][0:ocw, :, c0:c0 + 2 * TC])

            def alloc4(pool, T, halo, dt, nm, tag, zero_halo=True, n=4):
                ts = []
                for c in range(n):
                    t = pool.tile([128, B, T + 2 * halo], dt, name=f"{nm}{c}", tag=tag)
                    if halo and zero_halo:
                        nc.vector.memset(t[:, :, 0:halo], 0.0)
                        nc.vector.memset(t[:, :, halo + T:T + 2 * halo], 0.0)
                    ts.append(t)
                return ts

            # ---- input ----
            x0 = []
            for c in range(4):
                t = rpool.tile([128, B, T0 + 2], f16, name=f"x0_{c}", tag="r")
                nc.sync.dma_start(out=t[:, :, :], in_=xin[:, c, :, :])
                x0.append(t)

            x = alloc4(xpool, T0, 0, f32, "xin_conv", "x")
            conv("in", x0, 1, T0, dst=x, act=AF.Relu)

            # ---- res blocks + upsample ----
            T = T0
            for b in range(3):
                for ri, d in enumerate(DILS):
                    r = alloc4(rpool, T, d, f16, f"r{b}{ri}", "r")
                    for c in range(4):
                        nc.scalar.activation(r[c][:, :, d:d + T], x[c][:, :, :], AF.Relu)
                    rr = alloc4(rrpool, T, 0, f16, f"rr{b}{ri}", "rr")
                    conv(f"b{b}r{ri}c1", r, d, T, dst=rr, act=AF.Relu)
                    conv(f"b{b}r{ri}c2", rr, 0, T, stt_into=x)
                # upsample fused into even/odd stride-2 convs
                rup = alloc4(rpool, T, 1, f16, f"rup{b}", "r")
                for c in range(4):
                    nc.vector.tensor_copy(rup[c][:, :, 1:1 + T], x[c][:, :, :])
                last = (b == 2)
                T2 = 2 * T
                if last:
                    xn = alloc4(xpool, T2, 1, f16, f"xup{b}", "x")
                else:
                    xn = alloc4(xpool, T2, 0, f32, f"xup{b}", "x")
                conv(f"up{b}e", rup, 1, T, dst=xn, dst_halo=1 if last else 0,
                     act=AF.Identity, stride=2, suboff=0)
                conv(f"up{b}o", rup, 1, T, dst=xn, dst_halo=1 if last else 0,
                     act=AF.Identity, stride=2, suboff=1)
                x = xn
                T = T2

            # ---- out path (T=512), x is f16 with halo 1 ----
            r4 = alloc4(rpool, T, 1, f16, "r4", "r")
            conv("out1", x, 1, T, dst=r4, dst_halo=1, act=AF.Relu)
            y2 = alloc4(rrpool, T, 0, f16, "y2", "rr")
            conv("out2", r4, 1, T, dst=y2, act=AF.Identity)

            h = alloc4(rpool, T, 1, f16, "h", "r", n=2)
            conv("gcn", y2, 0, T, dst=h, dst_halo=1, act=AF.Identity)

            rows = (slice(0, 128), slice(128, U_FEATS))
            for cv, name in enumerate(("uconv", "lconv")):
                ots = [opool.tile([128, B, W], f32, name=f"o_{name}{oi}", tag="out")
                       for oi in range(2)]
                conv(name, h, 1, T, dst=ots, act=AF.Identity,
                     dma_to=[out[cv, rows[0], :, :], out[cv, rows[1], :, :]])

    _split_excess_waits(nc, mybir)
    return nc


def _get_nc():
    if not _NC_CACHE:
        _NC_CACHE.append(_build_nc())
    return _NC_CACHE[0]


# ----------------------------------------------------------------------------
# Entry point
# ----------------------------------------------------------------------------

def kernel(ux, lx, ubody_params, lbody_params, gcn_w, gcn_b,
           uconv_w, uconv_b, lconv_w, lconv_b):
    from concourse.bass_utils import run_bass_kernel_spmd

    def tonp(t):
        return np.asarray(t, np.float32)

    def tree(p):
        if isinstance(p, dict):
            return {k: tree(v) for k, v in p.items()}
        if isinstance(p, list):
            return [tree(v) for v in p]
        return tonp(p)

    ux = tonp(ux)
    lx = tonp(lx)
    ub, lb = tree(ubody_params), tree(lbody_params)
    gcn_w, gcn_b = tonp(gcn_w), tonp(gcn_b)
    uconv_w, uconv_b = tonp(uconv_w), tonp(uconv_b)
    lconv_w, lconv_b = tonp(lconv_w), tonp(lconv_b)

    nbatch = ux.shape[0]
    per = nbatch // 4  # samples per core (u on cores 0-3, l on 4-7)
    assert per == B

    wb_u, bb_u = _pack_core_weights(ub, gcn_w, gcn_b, uconv_w, uconv_b, lconv_w, lconv_b)
    wb_l, bb_l = _pack_core_weights(lb, gcn_w, gcn_b, uconv_w, uconv_b, lconv_w, lconv_b)

    in_maps = []
    for core in range(NCORES):
        branch_u = core < 4
        i0 = (core % 4) * B
        xs = ux[i0:i0 + B] if branch_u else lx[i0:i0 + B]
        in_maps.append({
            "xin": _pack_x(xs),
            "wb": wb_u if branch_u else wb_l,
            "bb": bb_u if branch_u else bb_l,
        })

    nc = _get_nc()
    res = run_bass_kernel_spmd(nc, in_maps, list(range(NCORES)))
    outs = [res.results[c]["out"] for c in range(NCORES)]

    ux_out = np.empty((nbatch, U_FEATS, W), np.float32)
    lx_out = np.empty((nbatch, L_FEATS, W), np.float32)
    for i in range(4):
        for bidx in range(B):
            s = i * B + bidx
            ux_out[s] = outs[i][0, :, bidx, :]
            lx_out[s] = outs[i + 4][1, :L_FEATS, bidx, :]
    # batch-0 GCN mix by linearity: uconv(0.5*(h_u0+h_l0)) = 0.5*(uconv(h_u0)+uconv(h_l0))
    ux_out[0] = 0.5 * (outs[0][0, :, 0, :] + outs[4][0, :, 0, :])
    lx_out[0] = 0.5 * (outs[0][1, :L_FEATS, 0, :] + outs[4][1, :L_FEATS, 0, :])
    return ux_out, lx_out


# revision 18
# speedup vs baseline: 1.1168x; 1.1143x over previous
"""Trainium2 Bass kernel for nn_Decoder_74938589381001.

Strategy (8 NeuronCores, SPMD single program):
  - Branch-parallel data split: cores 0-3 run the u-branch on 8 samples each,
    cores 4-7 run the l-branch on 8 samples each (branch weights are input
    data, so the program is identical across cores).
  - All convs are lowered to fp16 matmuls (fp32 PSUM accumulation, fp32
    residual stream in SBUF). Upsample+conv fuses to a pair of stride-2 k2
    convs. The degenerate batch-0 GCN mix is resolved on the host by
    linearity: every core computes BOTH final convs, and the host averages
    the sample-0 outputs of core 0 and core 4.
"""

import os
import numpy as np

# The bass kernel executes through the axon PJRT proxy; a cpu-pinned JAX env
# (sometimes set for reference runs) would hide the neuron devices.
if os.environ.get("JAX_PLATFORMS", "").strip().lower() == "cpu":
    os.environ.pop("JAX_PLATFORMS")

W = 512
B = 8            # samples per core
T0 = 64
DILS = (9, 3, 1)
NCORES = 8
TC = 64          # time-columns per matmul group (B*TC = 512 free dim)
U_FEATS = 156
L_FEATS = 107
GCN_OUT = 156
GCN_PAD = 256    # gcn rows padded so final convs contract 2 full chunks


def _layer_list():
    L = [("in", dict(n_ic=4, ocw=(128,) * 4, taps=(-1, 0, 1), dil=1))]
    for b in range(3):
        for ri, d in enumerate(DILS):
            L.append((f"b{b}r{ri}c1", dict(n_ic=4, ocw=(128,) * 4, taps=(-1, 0, 1), dil=d)))
            L.append((f"b{b}r{ri}c2", dict(n_ic=4, ocw=(128,) * 4, taps=(0,), dil=1)))
        L.append((f"up{b}e", dict(n_ic=4, ocw=(128,) * 4, taps=(-1, 0), dil=1)))
        L.append((f"up{b}o", dict(n_ic=4, ocw=(128,) * 4, taps=(0, 1), dil=1)))
    L += [
        ("out1", dict(n_ic=4, ocw=(128,) * 4, taps=(-1, 0, 1), dil=1)),
        # out2 (512->512 k3) composed with the GCN projection (512->156 1x1):
        # h = G@(W2*r4 + b2) + gb = (G@W2)*r4 + (G@b2 + gb)
        ("hconv", dict(n_ic=4, ocw=(128, 128), taps=(-1, 0, 1), dil=1)),
        # uconv and lconv merged into one 312-row conv (rows 0:156 = u, 156:263 = l)
        ("fconv", dict(n_ic=2, ocw=(128, 128, 56), taps=(-1, 0, 1), dil=1)),
    ]
    return L


LAYERS = _layer_list()
LSPEC = dict(LAYERS)


def _offsets():
    woff, boff = {}, {}
    wo = bo = 0
    for name, s in LAYERS:
        woff[name] = wo
        boff[name] = bo
        wo += len(s["ocw"]) * s["n_ic"] * len(s["taps"]) * 128
        bo += len(s["ocw"])
    return woff, boff, wo, bo


WOFF, BOFF, NWCOLS, NBCOLS = _offsets()


# ----------------------------------------------------------------------------
# Host-side packing
# ----------------------------------------------------------------------------

def _blocks(Wm, ocw_list, n_ic):
    """Wm: [Cout, Cin<=n_ic*128, K] fp32 -> lhsT blocks [slots, 128, 128] f16.

    slot index = (oc*n_ic + ic)*K + k; block[r, c] = Wm[oc0+c, ic0+r, k].
    """
    Cout, Cin, K = Wm.shape
    out = np.zeros((len(ocw_list) * n_ic * K, 128, 128), np.float16)
    s = 0
    for oi, ocw in enumerate(ocw_list):
        for ic in range(n_ic):
            icw = min(128, Cin - ic * 128)
            for k in range(K):
                blk = Wm[oi * 128:oi * 128 + ocw, ic * 128:ic * 128 + icw, k]
                out[s, :icw, :ocw] = blk.T.astype(np.float16)
                s += 1
    return out


def _bias_cols(bv, ocw_list):
    cols = np.zeros((128, len(ocw_list)), np.float32)
    for oi, ocw in enumerate(ocw_list):
        seg = bv[oi * 128:oi * 128 + ocw]
        cols[:len(seg), oi] = seg
    return cols


def _pack_core_weights(bp, gcn_w, gcn_b, uconv_w, uconv_b, lconv_w, lconv_b):
    """bp: branch params dict. Returns (wb [128, NWCOLS] f16, bb [128, NBCOLS] f32)."""
    wblocks, bcols = [], []

    def add(name, Wm, bv):
        s = LSPEC[name]
        assert Wm.shape[2] == len(s["taps"]), name
        wblocks.append(_blocks(Wm, s["ocw"], s["n_ic"]))
        bcols.append(_bias_cols(bv, s["ocw"]))

    add("in", bp["in_w"], bp["in_b"])
    for b in range(3):
        blk = bp["blocks"][b]
        for ri in range(3):
            rb = blk["res"][ri]
            add(f"b{b}r{ri}c1", rb["w1"], rb["b1"])
            add(f"b{b}r{ri}c2", rb["w2"], rb["b2"])
        w = blk["up_w"]
        We = np.stack([w[:, :, 0], w[:, :, 1] + w[:, :, 2]], axis=2)  # taps -1, 0
        Wo = np.stack([w[:, :, 0] + w[:, :, 1], w[:, :, 2]], axis=2)  # taps 0, +1
        add(f"up{b}e", We, blk["up_b"])
        add(f"up{b}o", Wo, blk["up_b"])
    add("out1", bp["out1_w"], bp["out1_b"])

    C = np.einsum('oc,cik->oik', gcn_w, bp["out2_w"])       # [156, 512, 3]
    Cb = gcn_w @ bp["out2_b"] + gcn_b
    Cp = np.zeros((GCN_PAD, W, 3), np.float32)
    Cp[:GCN_OUT] = C
    Cbp = np.zeros(GCN_PAD, np.float32)
    Cbp[:GCN_OUT] = Cb
    add("hconv", Cp, Cbp)

    F = np.zeros((2 * U_FEATS, GCN_PAD, 3), np.float32)     # [312, 256, 3]
    F[0:U_FEATS, :GCN_OUT] = uconv_w
    F[U_FEATS:U_FEATS + L_FEATS, :GCN_OUT] = lconv_w
    Fb = np.zeros(2 * U_FEATS, np.float32)
    Fb[0:U_FEATS] = uconv_b
    Fb[U_FEATS:U_FEATS + L_FEATS] = lconv_b
    add("fconv", F, Fb)

    blocks = np.concatenate(wblocks, axis=0)          # [slots, 128, 128]
    assert blocks.shape[0] * 128 == NWCOLS
    wb = np.ascontiguousarray(blocks.transpose(1, 0, 2).reshape(128, NWCOLS))
    bb = np.ascontiguousarray(np.concatenate(bcols, axis=1))
    assert bb.shape == (128, NBCOLS)
    return wb, bb


def _pack_x(xs):
    """xs: [B, 512, 64] fp32 -> [128, 4, B, 66] f16 with zeroed t-halo."""
    a = np.zeros((128, 4, B, T0 + 2), np.float16)
    xr = np.asarray(xs, np.float32).reshape(B, 4, 128, T0).astype(np.float16)
    a[:, :, :, 1:T0 + 1] = xr.transpose(2, 1, 0, 3)
    return a


# ----------------------------------------------------------------------------
# BIR post-pass: walrus rejects instructions with more than a couple of sync
# waits; split excess waits onto same-engine NOPs inserted just before.
# ----------------------------------------------------------------------------

def _split_excess_waits(nc, mybir, max_waits=1):
    ctr = 0
    for f in nc.m.functions:
        for blk in f.blocks:
            insts = blk.instructions
            i = 0
            while i < len(insts):
                inst = insts[i]
                si = inst.sync_info
                waits = list(si.on_wait) if si and si.on_wait else []
                if len(waits) > max_waits:
                    keep = waits[-max_waits:]
                    extra = waits[:-max_waits]
                    pos = i
                    for j in range(0, len(extra), max_waits):
                        chunk = extra[j:j + max_waits]
                        nop = mybir.InstNoOp(name=f"waitnop_{ctr}", ins=[], outs=[])
                        ctr += 1
                        nop.engine = inst.engine
                        nop.sync_info = mybir.SyncInfo(on_wait=list(chunk), on_update=[])
                        insts.insert(pos, nop)
                        pos += 1
                        i += 1
                    si.on_wait = list(keep)
                i += 1
    return ctr


# ----------------------------------------------------------------------------
# Device program
# ----------------------------------------------------------------------------

_NC_CACHE = []


def _build_nc():
    import concourse.bass as bass
    import concourse.tile as tile
    from concourse import mybir

    f16 = mybir.dt.float16
    f32 = mybir.dt.float32
    AF = mybir.ActivationFunctionType
    ADD = mybir.AluOpType.add

    nc = bass.Bass()
    xin = nc.declare_dram_parameter("xin", [128, 4, B, T0 + 2], f16, isOutput=False)
    wb = nc.declare_dram_parameter("wb", [128, NWCOLS], f16, isOutput=False)
    bbp = nc.declare_dram_parameter("bb", [128, NBCOLS], f32, isOutput=False)
    out = nc.declare_dram_parameter("out", [2 * U_FEATS, B, W], f32, isOutput=True)

    with tile.TileContext(nc) as tc:
        from contextlib import ExitStack
        with ExitStack() as ctx:
            wpool = ctx.enter_context(tc.tile_pool(name="wpool", bufs=3))
            xpool = ctx.enter_context(tc.tile_pool(name="xpool", bufs=5))
            rpool = ctx.enter_context(tc.tile_pool(name="rpool", bufs=6))
            rrpool = ctx.enter_context(tc.tile_pool(name="rrpool", bufs=4))
            opool = ctx.enter_context(tc.tile_pool(name="opool", bufs=3))
            spool = ctx.enter_context(tc.tile_pool(name="spool", bufs=1))
            ppool = ctx.enter_context(tc.tile_pool(name="ppool", bufs=6, space="PSUM"))

            bias_sb = spool.tile([128, NBCOLS], f32, name="bias_sb", tag="bias")
            nc.sync.dma_start(out=bias_sb[:, :], in_=bbp[:, :])

            def load_w(name):
                s = LSPEC[name]
                cols = len(s["ocw"]) * s["n_ic"] * len(s["taps"]) * 128
                wsb = wpool.tile([128, cols], f16, name=f"w_{name}", tag="w")
                if name == "in":
                    # split the first conv's weight load so the PE can start on
                    # oc-chunk 0 before the whole layer's weights have landed
                    q = cols // len(s["ocw"])
                    for i in range(len(s["ocw"])):
                        nc.sync.dma_start(
                            out=wsb[:, i * q:(i + 1) * q],
                            in_=wb[:, WOFF[name] + i * q:WOFF[name] + (i + 1) * q])
                else:
                    nc.sync.dma_start(out=wsb[:, :], in_=wb[:, WOFF[name]:WOFF[name] + cols])
                return wsb

            def conv(name, src, src_halo, T_in, *, dst=None, dst_halo=0,
                     act=None, stt_into=None, stride=1, suboff=0, groups=None,
                     dma_to=None):
                """One conv layer. src/dst: lists of [128, B, Tp] tiles.

                groups = number of TC-wide input column groups (T_in // TC).
                stride=2 writes outputs at dst positions dst_halo+suboff+2*t.
                """
                s = LSPEC[name]
                wsb = load_w(name)
                ng = groups if groups is not None else T_in // TC
                ntap = len(s["taps"])
                nmm = s["n_ic"] * ntap
                for g in range(ng):
                    for oi, ocw in enumerate(s["ocw"]):
                        ps = ppool.tile([128, B, TC], f32, name=f"ps_{name}_{g}_{oi}", tag="ps")
                        i = 0
                        for ic in range(s["n_ic"]):
                            for ki, tap in enumerate(s["taps"]):
                                slot = (oi * s["n_ic"] + ic) * ntap + ki
                                p0 = src_halo + g * TC + tap * s["dil"]
                                rhs = src[ic][:, :, p0:p0 + TC]
                                nc.tensor.matmul(
                                    ps[0:ocw, :, :],
                                    wsb[:, slot * 128: slot * 128 + ocw],
                                    rhs,
                                    start=(i == 0), stop=(i == nmm - 1),
                                )
                                i += 1
                        b_ap = bias_sb[0:ocw, BOFF[name] + oi: BOFF[name] + oi + 1]
                        if stt_into is not None:
                            xs = stt_into[oi][:, :, g * TC:(g + 1) * TC]
                            nc.vector.scalar_tensor_tensor(
                                out=xs, in0=ps[0:ocw, :, :], scalar=b_ap, in1=xs,
                                op0=ADD, op1=ADD)
                        else:
                            if stride == 1:
                                oap = dst[oi][0:ocw, :, dst_halo + g * TC: dst_halo + (g + 1) * TC]
                            else:
                                a = dst_halo + suboff + 2 * g * TC
                                oap = dst[oi][0:ocw, :, a:a + 2 * TC - 1:2]
                            nc.scalar.activation(oap, ps[0:ocw, :, :], act, bias=b_ap)
                            if dma_to is not None and g % 2 == 1:
                                c0 = (g - 1) * TC
                                nc.sync.dma_start(
                                    out=dma_to[oi][:, :, c0:c0 + 2 * TC],
                                    in_=dst[oi][0:ocw, :, c0:c0 + 2 * TC])

            def alloc4(pool, T, halo, dt, nm, tag, zero_halo=True, n=4):
                ts = []
                for c in range(n):
                    t = pool.tile([128, B, T + 2 * halo], dt, name=f"{nm}{c}", tag=tag)
                    if halo and zero_halo:
                        nc.vector.memset(t[:, :, 0:halo], 0.0)
                        nc.vector.memset(t[:, :, halo + T:T + 2 * halo], 0.0)
                    ts.append(t)
                return ts

            # ---- input ----
            x0 = []
            for c in range(4):
                t = rpool.tile([128, B, T0 + 2], f16, name=f"x0_{c}", tag="r")
                nc.sync.dma_start(out=t[:, :, :], in_=xin[:, c, :, :])
                x0.append(t)

            x = alloc4(xpool, T0, 0, f32, "xin_conv", "x")
            conv("in", x0, 1, T0, dst=x, act=AF.Relu)

            # ---- res blocks + upsample ----
            T = T0
            for b in range(3):
                for ri, d in enumerate(DILS):
                    r = alloc4(rpool, T, d, f16, f"r{b}{ri}", "r")
                    for c in range(4):
                        nc.scalar.activation(r[c][:, :, d:d + T], x[c][:, :, :], AF.Relu)
                    rr = alloc4(rrpool, T, 0, f16, f"rr{b}{ri}", "rr")
                    conv(f"b{b}r{ri}c1", r, d, T, dst=rr, act=AF.Relu)
                    conv(f"b{b}r{ri}c2", rr, 0, T, stt_into=x)
                # upsample fused into even/odd stride-2 convs
                rup = alloc4(rpool, T, 1, f16, f"rup{b}", "r")
                for c in range(4):
                    nc.vector.tensor_copy(rup[c][:, :, 1:1 + T], x[c][:, :, :])
                last = (b == 2)
                T2 = 2 * T
                if last:
                    xn = alloc4(xpool, T2, 1, f16, f"xup{b}", "x")
                else:
                    xn = alloc4(xpool, T2, 0, f32, f"xup{b}", "x")
                conv(f"up{b}e", rup, 1, T, dst=xn, dst_halo=1 if last else 0,
                     act=AF.Identity, stride=2, suboff=0)
                conv(f"up{b}o", rup, 1, T, dst=xn, dst_halo=1 if last else 0,
                     act=AF.Identity, stride=2, suboff=1)
                x = xn
                T = T2

            # ---- out path (T=512), x is f16 with halo 1 ----
            r4 = alloc4(rpool, T, 1, f16, "r4", "r")
            conv("out1", x, 1, T, dst=r4, dst_halo=1, act=AF.Relu)

            h = alloc4(rpool, T, 1, f16, "h", "r", n=2)
            conv("hconv", r4, 1, T, dst=h, dst_halo=1, act=AF.Identity)

            ocw_f = LSPEC["fconv"]["ocw"]
            ots = [opool.tile([128, B, W], f32, name=f"fo{oi}", tag="out")
                   for oi in range(3)]
            conv("fconv", h, 1, T, dst=ots, act=AF.Identity,
                 dma_to=[out[oi * 128:oi * 128 + ocw_f[oi], :, :]
                         for oi in range(3)])

    _split_excess_waits(nc, mybir)
    return nc


def _get_nc():
    if not _NC_CACHE:
        _NC_CACHE.append(_build_nc())
    return _NC_CACHE[0]


# ----------------------------------------------------------------------------
# Entry point
# ----------------------------------------------------------------------------

def kernel(ux, lx, ubody_params, lbody_params, gcn_w, gcn_b,
           uconv_w, uconv_b, lconv_w, lconv_b):
    from concourse.bass_utils import run_bass_kernel_spmd

    def tonp(t):
        return np.asarray(t, np.float32)

    def tree(p):
        if isinstance(p, dict):
            return {k: tree(v) for k, v in p.items()}
        if isinstance(p, list):
            return [tree(v) for v in p]
        return tonp(p)

    ux = tonp(ux)
    lx = tonp(lx)
    ub, lb = tree(ubody_params), tree(lbody_params)
    gcn_w, gcn_b = tonp(gcn_w), tonp(gcn_b)
    uconv_w, uconv_b = tonp(uconv_w), tonp(uconv_b)
    lconv_w, lconv_b = tonp(lconv_w), tonp(lconv_b)

    nbatch = ux.shape[0]
    per = nbatch // 4  # samples per core (u on cores 0-3, l on 4-7)
    assert per == B

    wb_u, bb_u = _pack_core_weights(ub, gcn_w, gcn_b, uconv_w, uconv_b, lconv_w, lconv_b)
    wb_l, bb_l = _pack_core_weights(lb, gcn_w, gcn_b, uconv_w, uconv_b, lconv_w, lconv_b)

    in_maps = []
    for core in range(NCORES):
        branch_u = core < 4
        i0 = (core % 4) * B
        xs = ux[i0:i0 + B] if branch_u else lx[i0:i0 + B]
        in_maps.append({
            "xin": _pack_x(xs),
            "wb": wb_u if branch_u else wb_l,
            "bb": bb_u if branch_u else bb_l,
        })

    nc = _get_nc()
    res = run_bass_kernel_spmd(nc, in_maps, list(range(NCORES)))
    outs = [res.results[c]["out"] for c in range(NCORES)]

    ux_out = np.empty((nbatch, U_FEATS, W), np.float32)
    lx_out = np.empty((nbatch, L_FEATS, W), np.float32)
    lo = U_FEATS
    for i in range(4):
        for bidx in range(B):
            s = i * B + bidx
            ux_out[s] = outs[i][:U_FEATS, bidx, :]
            lx_out[s] = outs[i + 4][lo:lo + L_FEATS, bidx, :]
    # batch-0 GCN mix by linearity: uconv(0.5*(h_u0+h_l0)) = 0.5*(uconv(h_u0)+uconv(h_l0))
    ux_out[0] = 0.5 * (outs[0][:U_FEATS, 0, :] + outs[4][:U_FEATS, 0, :])
    lx_out[0] = 0.5 * (outs[0][lo:lo + L_FEATS, 0, :] + outs[4][lo:lo + L_FEATS, 0, :])
    return ux_out, lx_out


# revision 24
# speedup vs baseline: 1.2373x; 1.1079x over previous
"""Trainium2 Bass kernel for nn_Decoder_74938589381001.

Strategy (8 NeuronCores, SPMD single program):
  - Branch-parallel data split: cores 0-3 run the u-branch on 8 samples each,
    cores 4-7 run the l-branch on 8 samples each (branch weights are input
    data, so the program is identical across cores).
  - All convs are lowered to fp16 matmuls (fp32 PSUM accumulation, fp32
    residual stream in SBUF). Upsample+conv fuses to a pair of stride-2 k2
    convs. The degenerate batch-0 GCN mix is resolved on the host by
    linearity: every core computes BOTH final convs, and the host averages
    the sample-0 outputs of core 0 and core 4.
"""

import os
import numpy as np

# The bass kernel executes through the axon PJRT proxy; a cpu-pinned JAX env
# (sometimes set for reference runs) would hide the neuron devices.
if os.environ.get("JAX_PLATFORMS", "").strip().lower() == "cpu":
    os.environ.pop("JAX_PLATFORMS")

W = 512
B = 8            # samples per core
T0 = 64
DILS = (9, 3, 1)
NCORES = 8
TC = 64          # time-columns per matmul group (B*TC = 512 free dim)
U_FEATS = 156
L_FEATS = 107
GCN_OUT = 156
GCN_PAD = 256    # gcn rows padded so final convs contract 2 full chunks


def _layer_list():
    L = [("in", dict(n_ic=4, ocw=(128,) * 4, taps=(-1, 0, 1), dil=1))]
    for b in range(3):
        for ri, d in enumerate(DILS):
            L.append((f"b{b}r{ri}c1", dict(n_ic=4, ocw=(128,) * 4, taps=(-1, 0, 1), dil=d)))
            L.append((f"b{b}r{ri}c2", dict(n_ic=4, ocw=(128,) * 4, taps=(0,), dil=1)))
        if b < 2:
            L.append((f"up{b}e", dict(n_ic=4, ocw=(128,) * 4, taps=(-1, 0), dil=1)))
            L.append((f"up{b}o", dict(n_ic=4, ocw=(128,) * 4, taps=(0, 1), dil=1)))
    L += [
        # up3 (k3 conv on nearest-x2-upsample) composed with out1 (k3): two
        # stride-2 k3 convs on the T=256 stream, one per output parity. The
        # composed taps are wrong at the two boundary output columns (they
        # assume y-pad = C/D/A/B-extrapolation instead of 0); outeE/outoE hold
        # the correction weights (one extra N=8 matmul per ic into the edge
        # column) and the edge-column biases.
        ("oute", dict(n_ic=4, ocw=(128,) * 4, taps=(-1, 0, 1), dil=1)),
        ("outeE", dict(n_ic=4, ocw=(128,) * 4, taps=(0,), dil=1)),
        ("outo", dict(n_ic=4, ocw=(128,) * 4, taps=(-1, 0, 1), dil=1)),
        ("outoE", dict(n_ic=4, ocw=(128,) * 4, taps=(0,), dil=1)),
        # out2 (512->512 k3) composed with the GCN projection (512->156 1x1):
        # h = G@(W2*r4 + b2) + gb = (G@W2)*r4 + (G@b2 + gb)
        ("hconv", dict(n_ic=4, ocw=(128, 128), taps=(-1, 0, 1), dil=1)),
        # uconv and lconv merged into one 312-row conv (rows 0:156 = u, 156:263 = l)
        ("fconv", dict(n_ic=2, ocw=(128, 128, 56), taps=(-1, 0, 1), dil=1)),
    ]
    return L


LAYERS = _layer_list()
LSPEC = dict(LAYERS)


def _offsets():
    woff, boff = {}, {}
    wo = bo = 0
    for name, s in LAYERS:
        woff[name] = wo
        boff[name] = bo
        wo += len(s["ocw"]) * s["n_ic"] * len(s["taps"]) * 128
        bo += len(s["ocw"])
    return woff, boff, wo, bo


WOFF, BOFF, NWCOLS, NBCOLS = _offsets()


# ----------------------------------------------------------------------------
# Host-side packing
# ----------------------------------------------------------------------------

def _blocks(Wm, ocw_list, n_ic):
    """Wm: [Cout, Cin<=n_ic*128, K] fp32 -> lhsT blocks [slots, 128, 128] f16.

    slot index = (oc*n_ic + ic)*K + k; block[r, c] = Wm[oc0+c, ic0+r, k].
    """
    Cout, Cin, K = Wm.shape
    out = np.zeros((len(ocw_list) * n_ic * K, 128, 128), np.float16)
    s = 0
    for oi, ocw in enumerate(ocw_list):
        for ic in range(n_ic):
            icw = min(128, Cin - ic * 128)
            for k in range(K):
                blk = Wm[oi * 128:oi * 128 + ocw, ic * 128:ic * 128 + icw, k]
                out[s, :icw, :ocw] = blk.T.astype(np.float16)
                s += 1
    return out


def _bias_cols(bv, ocw_list):
    cols = np.zeros((128, len(ocw_list)), np.float32)
    for oi, ocw in enumerate(ocw_list):
        seg = bv[oi * 128:oi * 128 + ocw]
        cols[:len(seg), oi] = seg
    return cols


def _pack_core_weights(bp, gcn_w, gcn_b, uconv_w, uconv_b, lconv_w, lconv_b):
    """bp: branch params dict. Returns (wb [128, NWCOLS] f16, bb [128, NBCOLS] f32)."""
    wblocks, bcols = [], []

    def add(name, Wm, bv):
        s = LSPEC[name]
        assert Wm.shape[2] == len(s["taps"]), name
        wblocks.append(_blocks(Wm, s["ocw"], s["n_ic"]))
        bcols.append(_bias_cols(bv, s["ocw"]))

    add("in", bp["in_w"], bp["in_b"])
    for b in range(3):
        blk = bp["blocks"][b]
        for ri in range(3):
            rb = blk["res"][ri]
            add(f"b{b}r{ri}c1", rb["w1"], rb["b1"])
            add(f"b{b}r{ri}c2", rb["w2"], rb["b2"])
        if b < 2:
            w = blk["up_w"]
            We = np.stack([w[:, :, 0], w[:, :, 1] + w[:, :, 2]], axis=2)  # taps -1, 0
            Wo = np.stack([w[:, :, 0] + w[:, :, 1], w[:, :, 2]], axis=2)  # taps 0, +1
            add(f"up{b}e", We, blk["up_b"])
            add(f"up{b}o", Wo, blk["up_b"])

    # --- up3 composed with out1 ---
    # y[2t] = A x[t-1] + B x[t];  y[2t+1] = C x[t] + D x[t+1]  (+ up_b each)
    # z[s]  = V0 y[s-1] + V1 y[s] + V2 y[s+1] + out1_b
    U = bp["blocks"][2]["up_w"]
    ub_ = bp["blocks"][2]["up_b"]
    V0, V1, V2 = bp["out1_w"][:, :, 0], bp["out1_w"][:, :, 1], bp["out1_w"][:, :, 2]
    vb = bp["out1_b"]
    A = U[:, :, 0]
    Bm = U[:, :, 1] + U[:, :, 2]
    Cm = U[:, :, 0] + U[:, :, 1]
    D = U[:, :, 2]

    def mm(a, b):
        return np.einsum('oc,ci->oi', a, b, optimize=True)

    We_ = np.stack([mm(V0, Cm) + mm(V1, A),
                    mm(V0, D) + mm(V1, Bm) + mm(V2, Cm),
                    mm(V2, D)], axis=2)
    Wo_ = np.stack([mm(V0, A),
                    mm(V0, Bm) + mm(V1, Cm) + mm(V2, A),
                    mm(V1, D) + mm(V2, Bm)], axis=2)
    b_int = vb + (V0 + V1 + V2) @ ub_
    add("oute", We_, b_int)
    # z[0]: composed form wrongly adds V0*(C x[-1] + D x[0]) for the true
    # y[-1] = 0; x[-1] = 0 via halo, so subtract V0@D x[0]. Edge bias: y[-1]
    # contributes no up_b -> out1_b + (V1+V2)@up_b.
    add("outeE", -(mm(V0, D))[:, :, None], vb + (V1 + V2) @ ub_)
    add("outo", Wo_, b_int)
    # z[2T-1]: composed form wrongly adds V2*(A x[T-1] + B x[T]); x[T] = 0 via
    # halo, so subtract V2@A x[T-1]. Edge bias: out1_b + (V0+V1)@up_b.
    add("outoE", -(mm(V2, A))[:, :, None], vb + (V0 + V1) @ ub_)

    C = np.einsum('oc,cik->oik', gcn_w, bp["out2_w"])       # [156, 512, 3]
    Cb = gcn_w @ bp["out2_b"] + gcn_b
    Cp = np.zeros((GCN_PAD, W, 3), np.float32)
    Cp[:GCN_OUT] = C
    Cbp = np.zeros(GCN_PAD, np.float32)
    Cbp[:GCN_OUT] = Cb
    add("hconv", Cp, Cbp)

    F = np.zeros((2 * U_FEATS, GCN_PAD, 3), np.float32)     # [312, 256, 3]
    F[0:U_FEATS, :GCN_OUT] = uconv_w
    F[U_FEATS:U_FEATS + L_FEATS, :GCN_OUT] = lconv_w
    Fb = np.zeros(2 * U_FEATS, np.float32)
    Fb[0:U_FEATS] = uconv_b
    Fb[U_FEATS:U_FEATS + L_FEATS] = lconv_b
    add("fconv", F, Fb)

    blocks = np.concatenate(wblocks, axis=0)          # [slots, 128, 128]
    assert blocks.shape[0] * 128 == NWCOLS
    wb = np.ascontiguousarray(blocks.transpose(1, 0, 2).reshape(128, NWCOLS))
    bb = np.ascontiguousarray(np.concatenate(bcols, axis=1))
    assert bb.shape == (128, NBCOLS)
    return wb, bb


def _pack_x(xs):
    """xs: [B, 512, 64] fp32 -> [128, 4, B, 66] f16 with zeroed t-halo."""
    a = np.zeros((128, 4, B, T0 + 2), np.float16)
    xr = np.asarray(xs, np.float32).reshape(B, 4, 128, T0).astype(np.float16)
    a[:, :, :, 1:T0 + 1] = xr.transpose(2, 1, 0, 3)
    return a


# ----------------------------------------------------------------------------
# BIR post-pass: walrus rejects instructions with more than a couple of sync
# waits; split excess waits onto same-engine NOPs inserted just before.
# ----------------------------------------------------------------------------

def _split_excess_waits(nc, mybir, max_waits=1):
    ctr = 0
    for f in nc.m.functions:
        for blk in f.blocks:
            insts = blk.instructions
            i = 0
            while i < len(insts):
                inst = insts[i]
                si = inst.sync_info
                waits = list(si.on_wait) if si and si.on_wait else []
                if len(waits) > max_waits:
                    keep = waits[-max_waits:]
                    extra = waits[:-max_waits]
                    pos = i
                    for j in range(0, len(extra), max_waits):
                        chunk = extra[j:j + max_waits]
                        nop = mybir.InstNoOp(name=f"waitnop_{ctr}", ins=[], outs=[])
                        ctr += 1
                        nop.engine = inst.engine
                        nop.sync_info = mybir.SyncInfo(on_wait=list(chunk), on_update=[])
                        insts.insert(pos, nop)
                        pos += 1
                        i += 1
                    si.on_wait = list(keep)
                i += 1
    return ctr


# ----------------------------------------------------------------------------
# Device program
# ----------------------------------------------------------------------------

_NC_CACHE = []


def _build_nc():
    import concourse.bass as bass
    import concourse.tile as tile
    from concourse import mybir

    f16 = mybir.dt.float16
    f32 = mybir.dt.float32
    AF = mybir.ActivationFunctionType
    ADD = mybir.AluOpType.add

    nc = bass.Bass()
    xin = nc.declare_dram_parameter("xin", [128, 4, B, T0 + 2], f16, isOutput=False)
    wb = nc.declare_dram_parameter("wb", [128, NWCOLS], f16, isOutput=False)
    bbp = nc.declare_dram_parameter("bb", [128, NBCOLS], f32, isOutput=False)
    out = nc.declare_dram_parameter("out", [2 * U_FEATS, B, W], f32, isOutput=True)

    with tile.TileContext(nc) as tc:
        from contextlib import ExitStack
        with ExitStack() as ctx:
            wpool = ctx.enter_context(tc.tile_pool(name="wpool", bufs=3))
            xpool = ctx.enter_context(tc.tile_pool(name="xpool", bufs=4))
            rpool = ctx.enter_context(tc.tile_pool(name="rpool", bufs=8))
            rrpool = ctx.enter_context(tc.tile_pool(name="rrpool", bufs=4))
            opool = ctx.enter_context(tc.tile_pool(name="opool", bufs=3))
            spool = ctx.enter_context(tc.tile_pool(name="spool", bufs=1))
            ppool = ctx.enter_context(tc.tile_pool(name="ppool", bufs=6, space="PSUM"))

            bias_sb = spool.tile([128, NBCOLS], f32, name="bias_sb", tag="bias")
            nc.sync.dma_start(out=bias_sb[:, :], in_=bbp[:, :])

            def load_w(name, tag="w"):
                s = LSPEC[name]
                cols = len(s["ocw"]) * s["n_ic"] * len(s["taps"]) * 128
                wsb = wpool.tile([128, cols], f16, name=f"w_{name}", tag=tag)
                if name == "in":
                    # split the first conv's weight load so the PE can start on
                    # oc-chunk 0 before the whole layer's weights have landed
                    q = cols // len(s["ocw"])
                    for i in range(len(s["ocw"])):
                        nc.sync.dma_start(
                            out=wsb[:, i * q:(i + 1) * q],
                            in_=wb[:, WOFF[name] + i * q:WOFF[name] + (i + 1) * q])
                else:
                    nc.sync.dma_start(out=wsb[:, :], in_=wb[:, WOFF[name]:WOFF[name] + cols])
                return wsb

            def conv(name, src, src_halo, T_in, *, dst=None, dst_halo=0,
                     act=None, stt_into=None, stride=1, suboff=0, groups=None,
                     dma_to=None, edge=None):
                """One conv layer. src/dst: lists of [128, B, Tp] tiles.

                groups = number of TC-wide input column groups (T_in // TC).
                stride=2 writes outputs at dst positions dst_halo+suboff+2*t.
                edge = (kind, wfix_tile, fixname): accumulate a correction
                matmul into the first/last output column and use the fix
                layer's bias there (for composed upsample+conv boundaries).
                """
                s = LSPEC[name]
                wsb = load_w(name)
                ng = groups if groups is not None else T_in // TC
                ntap = len(s["taps"])
                for g in range(ng):
                    for oi, ocw in enumerate(s["ocw"]):
                        ps = ppool.tile([128, B, TC], f32, name=f"ps_{name}_{g}_{oi}", tag="ps")
                        mms = []
                        for ic in range(s["n_ic"]):
                            for ki, tap in enumerate(s["taps"]):
                                slot = (oi * s["n_ic"] + ic) * ntap + ki
                                p0 = src_halo + g * TC + tap * s["dil"]
                                mms.append((wsb[:, slot * 128: slot * 128 + ocw],
                                            src[ic][:, :, p0:p0 + TC], None))
                        ecol = None
                        if edge is not None:
                            kind, wfix, fixname = edge
                            if kind == "first" and g == 0:
                                ecol = 0
                            elif kind == "last" and g == ng - 1:
                                ecol = TC - 1
                            if ecol is not None:
                                pfix = src_halo + g * TC + ecol
                                for ic in range(s["n_ic"]):
                                    fslot = oi * s["n_ic"] + ic
                                    mms.append((wfix[:, fslot * 128: fslot * 128 + ocw],
                                                src[ic][:, :, pfix:pfix + 1], ecol))
                        for i, (lh, rh, col) in enumerate(mms):
                            pap = ps[0:ocw, :, :] if col is None else ps[0:ocw, :, col:col + 1]
                            nc.tensor.matmul(pap, lh, rh,
                                             start=(i == 0), stop=(i == len(mms) - 1))
                        b_ap = bias_sb[0:ocw, BOFF[name] + oi: BOFF[name] + oi + 1]
                        if stt_into is not None:
                            xs = stt_into[oi][:, :, g * TC:(g + 1) * TC]
                            nc.vector.scalar_tensor_tensor(
                                out=xs, in0=ps[0:ocw, :, :], scalar=b_ap, in1=xs,
                                op0=ADD, op1=ADD)
                        elif ecol is not None:
                            # split epilogue: edge column gets the fix bias
                            be_ap = bias_sb[0:ocw, BOFF[fixname] + oi: BOFF[fixname] + oi + 1]
                            ae = dst_halo + suboff + 2 * (g * TC + ecol)
                            nc.scalar.activation(dst[oi][0:ocw, :, ae:ae + 1],
                                                 ps[0:ocw, :, ecol:ecol + 1], act, bias=be_ap)
                            if ecol == 0:
                                a0 = dst_halo + suboff + 2 * (g * TC + 1)
                                oap = dst[oi][0:ocw, :, a0:a0 + 2 * (TC - 1) - 1:2]
                                nc.scalar.activation(oap, ps[0:ocw, :, 1:TC], act, bias=b_ap)
                            else:
                                a0 = dst_halo + suboff + 2 * g * TC
                                oap = dst[oi][0:ocw, :, a0:a0 + 2 * (TC - 1) - 1:2]
                                nc.scalar.activation(oap, ps[0:ocw, :, 0:TC - 1], act, bias=b_ap)
                        else:
                            if stride == 1:
                                oap = dst[oi][0:ocw, :, dst_halo + g * TC: dst_halo + (g + 1) * TC]
                            else:
                                a = dst_halo + suboff + 2 * g * TC
                                oap = dst[oi][0:ocw, :, a:a + 2 * TC - 1:2]
                            nc.scalar.activation(oap, ps[0:ocw, :, :], act, bias=b_ap)
                            if dma_to is not None and g % 2 == 1:
                                c0 = (g - 1) * TC
                                nc.gpsimd.dma_start(
                                    out=dma_to[oi][:, :, c0:c0 + 2 * TC],
                                    in_=dst[oi][0:ocw, :, c0:c0 + 2 * TC])

            def alloc4(pool, T, halo, dt, nm, tag, zero_halo=True, n=4):
                ts = []
                for c in range(n):
                    t = pool.tile([128, B, T + 2 * halo], dt, name=f"{nm}{c}", tag=tag)
                    if halo and zero_halo:
                        nc.vector.memset(t[:, :, 0:halo], 0.0)
                        nc.vector.memset(t[:, :, halo + T:T + 2 * halo], 0.0)
                    ts.append(t)
                return ts

            # ---- input ----
            x0 = []
            for c in range(4):
                t = rpool.tile([128, B, T0 + 2], f16, name=f"x0_{c}", tag="r")
                nc.sync.dma_start(out=t[:, :, :], in_=xin[:, c, :, :])
                x0.append(t)

            x = alloc4(xpool, T0, 0, f32, "xin_conv", "x")
            conv("in", x0, 1, T0, dst=x, act=AF.Relu)

            # ---- res blocks + upsample ----
            T = T0
            for b in range(3):
                for ri, d in enumerate(DILS):
                    r = alloc4(rpool, T, d, f16, f"r{b}{ri}", "r")
                    for c in range(4):
                        nc.scalar.activation(r[c][:, :, d:d + T], x[c][:, :, :], AF.Relu)
                    rr = alloc4(rrpool, T, 0, f16, f"rr{b}{ri}", "rr")
                    conv(f"b{b}r{ri}c1", r, d, T, dst=rr, act=AF.Relu)
                    conv(f"b{b}r{ri}c2", rr, 0, T, stt_into=x)
                if b < 2:
                    # upsample fused into even/odd stride-2 convs
                    rup = alloc4(rpool, T, 1, f16, f"rup{b}", "r")
                    for c in range(4):
                        nc.vector.tensor_copy(rup[c][:, :, 1:1 + T], x[c][:, :, :])
                    T2 = 2 * T
                    xn = alloc4(xpool, T2, 0, f32, f"xup{b}", "x")
                    conv(f"up{b}e", rup, 1, T, dst=xn, act=AF.Identity,
                         stride=2, suboff=0)
                    conv(f"up{b}o", rup, 1, T, dst=xn, act=AF.Identity,
                         stride=2, suboff=1)
                    x = xn
                    T = T2

            # ---- composed up3+out1: two stride-2 k3 convs, T=256 -> r4 @ T=512
            rup2 = alloc4(rpool, T, 1, f16, "rup2", "r")
            for c in range(4):
                nc.vector.tensor_copy(rup2[c][:, :, 1:1 + T], x[c][:, :, :])
            T2 = 2 * T
            r4 = alloc4(rpool, T2, 1, f16, "r4", "r")
            wfixE = load_w("outeE", tag="wfix")
            wfixO = load_w("outoE", tag="wfix")
            conv("oute", rup2, 1, T, dst=r4, dst_halo=1, act=AF.Relu,
                 stride=2, suboff=0, edge=("first", wfixE, "outeE"))
            conv("outo", rup2, 1, T, dst=r4, dst_halo=1, act=AF.Relu,
                 stride=2, suboff=1, edge=("last", wfixO, "outoE"))
            T = T2

            h = alloc4(rpool, T, 1, f16, "h", "r", n=2)
            conv("hconv", r4, 1, T, dst=h, dst_halo=1, act=AF.Identity)

            ocw_f = LSPEC["fconv"]["ocw"]
            ots = [opool.tile([128, B, W], f16, name=f"fo{oi}", tag="out")
                   for oi in range(3)]
            conv("fconv", h, 1, T, dst=ots, act=AF.Identity,
                 dma_to=[out[oi * 128:oi * 128 + ocw_f[oi], :, :]
                         for oi in range(3)])

    _split_excess_waits(nc, mybir)
    return nc


def _get_nc():
    if not _NC_CACHE:
        _NC_CACHE.append(_build_nc())
    return _NC_CACHE[0]


# ----------------------------------------------------------------------------
# Entry point
# ----------------------------------------------------------------------------

def kernel(ux, lx, ubody_params, lbody_params, gcn_w, gcn_b,
           uconv_w, uconv_b, lconv_w, lconv_b):
    from concourse.bass_utils import run_bass_kernel_spmd

    def tonp(t):
        return np.asarray(t, np.float32)

    def tree(p):
        if isinstance(p, dict):
            return {k: tree(v) for k, v in p.items()}
        if isinstance(p, list):
            return [tree(v) for v in p]
        return tonp(p)

    ux = tonp(ux)
    lx = tonp(lx)
    ub, lb = tree(ubody_params), tree(lbody_params)
    gcn_w, gcn_b = tonp(gcn_w), tonp(gcn_b)
    uconv_w, uconv_b = tonp(uconv_w), tonp(uconv_b)
    lconv_w, lconv_b = tonp(lconv_w), tonp(lconv_b)

    nbatch = ux.shape[0]
    per = nbatch // 4  # samples per core (u on cores 0-3, l on 4-7)
    assert per == B

    wb_u, bb_u = _pack_core_weights(ub, gcn_w, gcn_b, uconv_w, uconv_b, lconv_w, lconv_b)
    wb_l, bb_l = _pack_core_weights(lb, gcn_w, gcn_b, uconv_w, uconv_b, lconv_w, lconv_b)

    in_maps = []
    for core in range(NCORES):
        branch_u = core < 4
        i0 = (core % 4) * B
        xs = ux[i0:i0 + B] if branch_u else lx[i0:i0 + B]
        in_maps.append({
            "xin": _pack_x(xs),
            "wb": wb_u if branch_u else wb_l,
            "bb": bb_u if branch_u else bb_l,
        })

    nc = _get_nc()
    res = run_bass_kernel_spmd(nc, in_maps, list(range(NCORES)))
    outs = [res.results[c]["out"] for c in range(NCORES)]

    ux_out = np.empty((nbatch, U_FEATS, W), np.float32)
    lx_out = np.empty((nbatch, L_FEATS, W), np.float32)
    lo = U_FEATS
    for i in range(4):
        for bidx in range(B):
            s = i * B + bidx
            ux_out[s] = outs[i][:U_FEATS, bidx, :]
            lx_out[s] = outs[i + 4][lo:lo + L_FEATS, bidx, :]
    # batch-0 GCN mix by linearity: uconv(0.5*(h_u0+h_l0)) = 0.5*(uconv(h_u0)+uconv(h_l0))
    ux_out[0] = 0.5 * (outs[0][:U_FEATS, 0, :] + outs[4][:U_FEATS, 0, :])
    lx_out[0] = 0.5 * (outs[0][lo:lo + L_FEATS, 0, :] + outs[4][lo:lo + L_FEATS, 0, :])
    return ux_out, lx_out
